# revision 8
# baseline (speedup 1.0000x reference)
"""TRN2 Bass kernel for nn_Attention_369367187796.

Reference computation (B=4, DX=1024, N=4096, DQ=DK=DV=1024, fp32):
    Q = Wq @ x[b]; K = Wk @ x[b]; V = Wv @ x[b]          (per batch)
    scores = Q @ K.T   (contract n)
    p = softmax(scores / sqrt(DQ), axis=q)               <- softmax over q!
    out = p.T-contracted: out[q,n] = sum_k p[q,k] V[k,n]

Sharding: 8 cores = 4 batches x 2 dk-halves. Each core computes, for its
(batch b, k-half h): the full Q, its half of K and V, scoresT[k_half, q]
(softmax over q is the free axis -> fully local), and the partial
out[q, n] = sum_{k in half} p[k,q] V[k,n]. Host sums the two partials.

Precision strategy (HW-validated):
  - float32r (fp32 rounded to 11 mantissa bits) matmuls run at full PE rate;
    native fp32 runs at 1/4 rate; bf16 alone flips softmax argmaxes (logits
    have std ~740 with top-2 gaps down to ~0.4).
  - Q/K projections: W split into f32r hi+lo (2 passes) - W's positive mean
    makes its rounding error coherent over the contraction; x single f32r.
  - scores: Q/K evicted as bf16 hi+lo, 3-pass split matmul (hh, hl, lh).
  - V projection and p@V: single-pass f32r (errors pass through softmax
    un-amplified). End-to-end rel err vs fp64 reference ~1e-4.

Layouts (per core):
  QT (n, q) and KT (n, k) are computed transposed so the scores matmul
  contracts n on partitions and softmax lands on the free axis:
    QT[n,q] = sum_d x[d,n] WqT[d,q]   lhsT = x-tile [d,n], rhs = WqT [d,q]
    scoresT[k,q]: lhsT = KT [n,k], rhs = QT [n,q]
    out[q,n]:     lhsT = pT [k,q],  rhs = V  [k,n]
  QT/KT spill to DRAM between projection and scores phases; V spills as
  fp32 and is re-rounded to f32r on reload (the walrus verifier requires
  f32r matmul operands to be produced by a rounding compute op, not DMA).
"""

import math

import numpy as np

B_FULL, DX_FULL, N_FULL = 4, 1024, 4096
DQ_FULL = DK_FULL = 1024
N_CORES = 8

# precision mode for the Q/K projections: "f32r" (1 pass), "wsplit" (2),
# "bothsplit" (3)
QK_PROJ_MODE = "wsplit"
# scores: "bf3" = 3-pass bf16 hi/lo split
SCORES_MODE = "bf3"


def _build_core_kernel(DX, N, DQ, DKH, qk_mode=QK_PROJ_MODE, bench=False):
    import concourse.bass as bass
    import concourse.mybir as mybir
    import concourse.tile as tile
    from concourse import bacc

    f32 = mybir.dt.float32
    f32r = mybir.dt.float32r
    bf16 = mybir.dt.bfloat16

    P = 128
    DT = DX // P          # d-tiles (contraction tiles for projections)
    NT512 = N // 512      # n chunks of 512
    NT128 = N // P        # n tiles of 128
    QC = (DQ + 511) // 512  # q chunks of <=512
    QCS = min(DQ, 512)
    KT = DKH // P         # k tiles of 128
    QT128 = DQ // P       # q tiles (out partitions)
    scale = 1.0 / math.sqrt(DQ)

    assert DX % P == 0 and N % 512 == 0 and DQ % P == 0 and DKH % P == 0
    assert DQ % QCS == 0

    nc = bacc.Bacc(None, target_bir_lowering=False, debug=False)

    kind_big = "Internal" if bench else "ExternalInput"
    kind_out = "Internal" if bench else "ExternalOutput"
    xb = nc.dram_tensor("xb", [DX, N], f32, kind=kind_big)
    wqt = nc.dram_tensor("wqt", [DX, DQ], f32, kind=kind_big)
    wkt = nc.dram_tensor("wkt", [DX, DKH], f32, kind=kind_big)
    wvt = nc.dram_tensor("wvt", [DX, DKH], f32, kind=kind_big)
    # tiny input consumed into one output element (value 0 at rest): lets a
    # benchmark chain data dependencies between repeated NEFF executions
    seed = nc.dram_tensor("seed", [1, 1], f32, kind="ExternalInput")
    out = nc.dram_tensor("out", [DQ, N], f32, kind=kind_out)
    sink = nc.dram_tensor("sink", [1, 1], f32, kind="ExternalOutput") if bench else None

    xv = xb.ap().rearrange("(dt p) n -> p dt n", p=P)
    wqv = wqt.ap().rearrange("(dt p) q -> p dt q", p=P)
    wkv = wkt.ap().rearrange("(dt p) k -> p dt k", p=P)
    wvv = wvt.ap().rearrange("(dt p) k -> p dt k", p=P)

    with tile.TileContext(nc) as tc:
        with (
            tc.tile_pool(name="dram", bufs=1, space="DRAM") as dram,
            tc.tile_pool(name="ps", bufs=8, space="PSUM") as ps,
        ):
            # DRAM spill tensors
            qth_d = dram.tile([N, DQ], bf16)
            qtl_d = dram.tile([N, DQ], bf16)
            kth_d = dram.tile([N, DKH], bf16)
            ktl_d = dram.tile([N, DKH], bf16)
            vsp_d = dram.tile([DKH, N], f32)

            # ---------------- Phase 0 + 1: projections ----------------
            with (
                tc.tile_pool(name="pw", bufs=1) as pw,
                tc.tile_pool(name="pwstage", bufs=2) as pwstage,
                tc.tile_pool(name="px", bufs=2) as px,
                tc.tile_pool(name="pev", bufs=2) as pev,
            ):
                # --- weight prep ---
                if qk_mode == "f32r":
                    wq_h = pw.tile([P, DT, DQ], f32r, tag="wqh")
                    wk_h = pw.tile([P, DT, DKH], f32r, tag="wkh")
                    wq_l = wk_l = None
                else:
                    wq_h = pw.tile([P, DT, DQ], f32r, tag="wqh")
                    wq_l = pw.tile([P, DT, DQ], f32r, tag="wql")
                    wk_h = pw.tile([P, DT, DKH], f32r, tag="wkh")
                    wk_l = pw.tile([P, DT, DKH], f32r, tag="wkl")
                wv_r = pw.tile([P, DT, DKH], f32r, tag="wvr")

                for dt in range(DT):
                    wtmp = pwstage.tile([P, DQ], f32, tag="wtmp")
                    nc.sync.dma_start(wtmp[:], wqv[:, dt])
                    nc.vector.tensor_copy(wq_h[:, dt], wtmp[:])
                    if wq_l is not None:
                        nc.vector.tensor_sub(wq_l[:, dt], wtmp[:], wq_h[:, dt])

                    wtmp2 = pwstage.tile([P, DKH], f32, tag="wtmp2")
                    nc.sync.dma_start(wtmp2[:], wkv[:, dt])
                    nc.vector.tensor_copy(wk_h[:, dt], wtmp2[:])
                    if wk_l is not None:
                        nc.vector.tensor_sub(wk_l[:, dt], wtmp2[:], wk_h[:, dt])

                    wtmp3 = pwstage.tile([P, DKH], f32, tag="wtmp3")
                    nc.sync.dma_start(wtmp3[:], wvv[:, dt])
                    nc.vector.tensor_copy(wv_r[:, dt], wtmp3[:])

                # --- x chunks: project ---
                for c in range(NT512):
                    ncol = bass.ds(c * 512, 512)
                    xc = px.tile([P, DT, 512], f32, tag="xc")
                    nc.sync.dma_start(xc[:], xv[:, :, ncol])
                    xr = px.tile([P, DT, 512], f32r, tag="xr")
                    nc.vector.tensor_copy(xr[:], xc[:])
                    if qk_mode == "bothsplit":
                        xl = px.tile([P, DT, 512], f32r, tag="xl")
                        nc.vector.tensor_sub(xl[:], xc[:], xr[:])

                    # V projection: psum [v-128, n-512]
                    for vt in range(KT):
                        vps = ps.tile([P, 512], f32, tag="ps")
                        vsl = bass.ds(vt * P, P)
                        for dt in range(DT):
                            nc.tensor.matmul(
                                vps[:], wv_r[:, dt, vsl], xr[:, dt],
                                start=(dt == 0), stop=(dt == DT - 1),
                            )
                        vsb = pev.tile([P, 512], f32, tag="vsb")
                        nc.vector.tensor_copy(vsb[:], vps[:])
                        nc.sync.dma_start(vsp_d[vt * P:(vt + 1) * P, ncol], vsb[:])

                    # QT / KT projections per n-subtile
                    for nt in range(4):
                        gnt = c * 4 + nt   # global n-128 tile
                        xsl = bass.ds(nt * P, P)
                        for qc in range(QC):
                            qsl = bass.ds(qc * QCS, QCS)
                            qps = ps.tile([P, QCS], f32, tag="ps")
                            first = True
                            for dt in range(DT):
                                if qk_mode == "f32r":
                                    nc.tensor.matmul(
                                        qps[:], xr[:, dt, xsl], wq_h[:, dt, qsl],
                                        start=first, stop=(dt == DT - 1))
                                    first = False
                                elif qk_mode == "wsplit":
                                    nc.tensor.matmul(
                                        qps[:], xr[:, dt, xsl], wq_h[:, dt, qsl],
                                        start=first, stop=False)
                                    nc.tensor.matmul(
                                        qps[:], xr[:, dt, xsl], wq_l[:, dt, qsl],
                                        start=False, stop=(dt == DT - 1))
                                    first = False
                                else:  # bothsplit
                                    nc.tensor.matmul(
                                        qps[:], xr[:, dt, xsl], wq_h[:, dt, qsl],
                                        start=first, stop=False)
                                    nc.tensor.matmul(
                                        qps[:], xr[:, dt, xsl], wq_l[:, dt, qsl],
                                        start=False, stop=False)
                                    nc.tensor.matmul(
                                        qps[:], xl[:, dt, xsl], wq_h[:, dt, qsl],
                                        start=False, stop=(dt == DT - 1))
                                    first = False
                            qh = pev.tile([P, QCS], bf16, tag="qh")
                            ql = pev.tile([P, QCS], bf16, tag="ql")
                            nc.vector.tensor_copy(qh[:], qps[:])
                            nc.vector.tensor_sub(ql[:], qps[:], qh[:])
                            rows = bass.ds(gnt * P, P)
                            nc.sync.dma_start(qth_d[rows, qsl], qh[:])
                            nc.sync.dma_start(qtl_d[rows, qsl], ql[:])

                        kps = ps.tile([P, DKH], f32, tag="ps")
                        first = True
                        for dt in range(DT):
                            if qk_mode == "f32r":
                                nc.tensor.matmul(
                                    kps[:], xr[:, dt, xsl], wk_h[:, dt],
                                    start=first, stop=(dt == DT - 1))
                                first = False
                            elif qk_mode == "wsplit":
                                nc.tensor.matmul(
                                    kps[:], xr[:, dt, xsl], wk_h[:, dt],
                                    start=first, stop=False)
                                nc.tensor.matmul(
                                    kps[:], xr[:, dt, xsl], wk_l[:, dt],
                                    start=False, stop=(dt == DT - 1))
                                first = False
                            else:
                                nc.tensor.matmul(
                                    kps[:], xr[:, dt, xsl], wk_h[:, dt],
                                    start=first, stop=False)
                                nc.tensor.matmul(
                                    kps[:], xr[:, dt, xsl], wk_l[:, dt],
                                    start=False, stop=False)
                                nc.tensor.matmul(
                                    kps[:], xl[:, dt, xsl], wk_h[:, dt],
                                    start=False, stop=(dt == DT - 1))
                                first = False
                        kh = pev.tile([P, DKH], bf16, tag="kh")
                        kl = pev.tile([P, DKH], bf16, tag="kl")
                        nc.vector.tensor_copy(kh[:], kps[:])
                        nc.vector.tensor_sub(kl[:], kps[:], kh[:])
                        rows = bass.ds(gnt * P, P)
                        nc.sync.dma_start(kth_d[rows, :], kh[:])
                        nc.sync.dma_start(ktl_d[rows, :], kl[:])

            # ---------------- Phase 2: scoresT + softmax ----------------
            pres_cm = tc.tile_pool(name="resident", bufs=1)
            pres = pres_cm.__enter__()
            scores_sb = [pres.tile([P, DQ], f32, tag=f"sc{kt}", name=f"scores{kt}") for kt in range(KT)]
            p_r = [pres.tile([P, DQ], f32r, tag=f"pr{kt}", name=f"p{kt}") for kt in range(KT)]

            with (
                tc.tile_pool(name="pstream", bufs=3) as pstream,
                tc.tile_pool(name="psmx", bufs=2) as psmx,
                tc.tile_pool(name="pstat", bufs=2) as pstat,
            ):
                for qc in range(QC):
                    qsl = bass.ds(qc * QCS, QCS)
                    s_ps = [ps.tile([P, QCS], f32, tag="ps", name=f"sps{qc}_{i}") for i in range(KT)]
                    for nt in range(NT128):
                        rows = bass.ds(nt * P, P)
                        qh_s = pstream.tile([P, QCS], bf16, tag="qh_s")
                        ql_s = pstream.tile([P, QCS], bf16, tag="ql_s")
                        kh_s = pstream.tile([P, DKH], bf16, tag="kh_s")
                        kl_s = pstream.tile([P, DKH], bf16, tag="kl_s")
                        nc.sync.dma_start(qh_s[:], qth_d[rows, qsl])
                        nc.sync.dma_start(ql_s[:], qtl_d[rows, qsl])
                        nc.sync.dma_start(kh_s[:], kth_d[rows, :])
                        nc.sync.dma_start(kl_s[:], ktl_d[rows, :])
                        for kt in range(KT):
                            ksl = bass.ds(kt * P, P)
                            nc.tensor.matmul(
                                s_ps[kt][:], kh_s[:, ksl], qh_s[:],
                                start=(nt == 0), stop=False)
                            nc.tensor.matmul(
                                s_ps[kt][:], kh_s[:, ksl], ql_s[:],
                                start=False, stop=False)
                            nc.tensor.matmul(
                                s_ps[kt][:], kl_s[:, ksl], qh_s[:],
                                start=False, stop=(nt == NT128 - 1))
                    for kt in range(KT):
                        nc.vector.tensor_copy(scores_sb[kt][:, qsl], s_ps[kt][:])

                # softmax over q (free axis) per k row; fold in 1/sqrt(DQ)
                for kt in range(KT):
                    m = pstat.tile([P, 1], f32, tag="m")
                    negm = pstat.tile([P, 1], f32, tag="negm")
                    den = pstat.tile([P, 1], f32, tag="den")
                    rden = pstat.tile([P, 1], f32, tag="rden")
                    nc.vector.reduce_max(m[:], scores_sb[kt][:],
                                         axis=mybir.AxisListType.X)
                    nc.vector.tensor_scalar_mul(negm[:], m[:], -scale)
                    e = psmx.tile([P, DQ], f32, tag="e")
                    nc.scalar.activation(
                        e[:], scores_sb[kt][:],
                        mybir.ActivationFunctionType.Exp,
                        bias=negm[:], scale=scale, accum_out=den[:])
                    nc.vector.reciprocal(rden[:], den[:])
                    nc.vector.tensor_scalar_mul(p_r[kt][:], e[:], rden[:])

            # ---------------- Phase 3: out = pT.T @ V ----------------
            with (
                tc.tile_pool(name="pvin", bufs=KT + 2) as pvin,
                tc.tile_pool(name="pout", bufs=4) as pout,
                tc.tile_pool(name="pseed", bufs=1) as pseed,
            ):
                seed_sb = pseed.tile([1, 1], f32, tag="seed")
                nc.sync.dma_start(seed_sb[:], seed.ap())
                for c in range(NT512):
                    ncol = bass.ds(c * 512, 512)
                    vrr = []
                    for kt in range(KT):
                        vf = pvin.tile([P, 512], f32, tag="vf")
                        nc.sync.dma_start(vf[:], vsp_d[kt * P:(kt + 1) * P, ncol])
                        vr = pvin.tile([P, 512], f32r, tag="vr")
                        nc.vector.tensor_copy(vr[:], vf[:])
                        vrr.append(vr)
                    for qt in range(QT128):
                        ops = ps.tile([P, 512], f32, tag="ps")
                        qsl2 = bass.ds(qt * P, P)
                        for kt in range(KT):
                            nc.tensor.matmul(
                                ops[:], p_r[kt][:, qsl2], vrr[kt][:],
                                start=(kt == 0), stop=(kt == KT - 1))
                        osb = pout.tile([P, 512], f32, tag="osb")
                        nc.vector.tensor_copy(osb[:], ops[:])
                        if c == 0 and qt == 0:
                            nc.vector.tensor_scalar_add(
                                osb[0:1, 0:1], ops[0:1, 0:1], seed_sb[:])
                            if sink is not None:
                                nc.sync.dma_start(sink.ap(), osb[0:1, 0:1])
                        nc.sync.dma_start(out.ap()[qt * P:(qt + 1) * P, ncol], osb[:])
            pres_cm.__exit__(None, None, None)

    nc.compile()
    return nc


_CACHE = {}


def _get_nc(DX, N, DQ, DKH, qk_mode=QK_PROJ_MODE):
    key = (DX, N, DQ, DKH, qk_mode)
    if key not in _CACHE:
        _CACHE[key] = _build_core_kernel(DX, N, DQ, DKH, qk_mode)
    return _CACHE[key]


def _run(x, Wq, Wk, Wv, **spmd_kwargs):
    """Run the SPMD kernel; returns (out, BassKernelResults)."""
    from concourse.bass_utils import run_bass_kernel_spmd

    B, DX, N = x.shape
    DQ = Wq.shape[0]
    DK = Wk.shape[0]
    assert (B, DX, N, DQ, DK) == (B_FULL, DX_FULL, N_FULL, DQ_FULL, DK_FULL)
    DKH = DK // 2

    nc = _get_nc(DX, N, DQ, DKH)

    WqT = np.ascontiguousarray(Wq.T, dtype=np.float32)
    WkT = np.ascontiguousarray(Wk.T, dtype=np.float32)
    WvT = np.ascontiguousarray(Wv.T, dtype=np.float32)

    in_maps = []
    for c in range(N_CORES):
        b, h = divmod(c, 2)
        hsl = slice(h * DKH, (h + 1) * DKH)
        in_maps.append({
            "xb": np.ascontiguousarray(x[b], dtype=np.float32),
            "wqt": WqT,
            "wkt": np.ascontiguousarray(WkT[:, hsl]),
            "wvt": np.ascontiguousarray(WvT[:, hsl]),
            "seed": np.zeros((1, 1), np.float32),
        })

    res = run_bass_kernel_spmd(nc, in_maps, core_ids=list(range(N_CORES)),
                               **spmd_kwargs)
    out = np.empty((B, DQ, N), np.float32)
    for b in range(B):
        out[b] = res.results[2 * b]["out"] + res.results[2 * b + 1]["out"]
    return out, res


def kernel(x, Wq, Wk, Wv):
    return _run(x, Wq, Wk, Wv)[0]


# revision 9
# speedup vs baseline: 5.7304x; 5.7304x over previous
"""TRN2 Bass kernel for nn_Attention_369367187796.

Reference computation (B=4, DX=1024, N=4096, DQ=DK=DV=1024, fp32):
    Q = Wq @ x[b]; K = Wk @ x[b]; V = Wv @ x[b]          (per batch)
    scores = Q @ K.T   (contract n)
    p = softmax(scores / sqrt(DQ), axis=q)               <- softmax over q!
    out = p.T-contracted: out[q,n] = sum_k p[q,k] V[k,n]

Sharding: 8 cores = 4 batches x 2 dk-halves. Each core computes, for its
(batch b, k-half h): the full Q, its half of K and V, scoresT[k_half, q]
(softmax over q is the free axis -> fully local), and the partial
out[q, n] = sum_{k in half} p[k,q] V[k,n]. Host sums the two partials.

Precision strategy (HW-validated):
  - float32r (fp32 rounded to 11 mantissa bits) matmuls run at full PE rate;
    native fp32 runs at 1/4 rate; bf16 alone flips softmax argmaxes (logits
    have std ~740 with top-2 gaps down to ~0.4).
  - Q/K projections: W split into f32r hi+lo (2 passes) - W's positive mean
    makes its rounding error coherent over the contraction; x single f32r.
  - scores: Q/K evicted as bf16 hi+lo, 3-pass split matmul (hh, hl, lh).
  - V projection and p@V: single-pass f32r (errors pass through softmax
    un-amplified). End-to-end rel err vs fp64 reference ~1e-4.

Layouts (per core):
  QT (n, q) and KT (n, k) are computed transposed so the scores matmul
  contracts n on partitions and softmax lands on the free axis:
    QT[n,q] = sum_d x[d,n] WqT[d,q]   lhsT = x-tile [d,n], rhs = WqT [d,q]
    scoresT[k,q]: lhsT = KT [n,k], rhs = QT [n,q]
    out[q,n]:     lhsT = pT [k,q],  rhs = V  [k,n]
  QT/KT spill to DRAM between projection and scores phases; V spills as
  fp32 and is re-rounded to f32r on reload (the walrus verifier requires
  f32r matmul operands to be produced by a rounding compute op, not DMA).
"""

import math

import numpy as np

B_FULL, DX_FULL, N_FULL = 4, 1024, 4096
DQ_FULL = DK_FULL = 1024
N_CORES = 8

# precision mode for the Q/K projections: "f32r" (1 pass), "wsplit" (2),
# "bothsplit" (3)
QK_PROJ_MODE = "wsplit"
# scores: "bf3" = 3-pass bf16 hi/lo split
SCORES_MODE = "bf3"


def _build_core_kernel(DX, N, DQ, DKH, qk_mode=QK_PROJ_MODE, bench=False, bench_reps=0):
    import concourse.bass as bass
    import concourse.mybir as mybir
    import concourse.tile as tile
    from concourse import bacc

    f32 = mybir.dt.float32
    f32r = mybir.dt.float32r
    bf16 = mybir.dt.bfloat16

    P = 128
    DT = DX // P          # d-tiles (contraction tiles for projections)
    NT512 = N // 512      # n chunks of 512
    NT128 = N // P        # n tiles of 128
    QC = (DQ + 511) // 512  # q chunks of <=512
    QCS = min(DQ, 512)
    KT = DKH // P         # k tiles of 128
    QT128 = DQ // P       # q tiles (out partitions)
    scale = 1.0 / math.sqrt(DQ)

    assert DX % P == 0 and N % 512 == 0 and DQ % P == 0 and DKH % P == 0
    assert DQ % QCS == 0

    nc = bacc.Bacc(None, target_bir_lowering=False, debug=False)

    kind_big = "Internal" if bench else "ExternalInput"
    kind_out = "Internal" if bench else "ExternalOutput"
    xb = nc.dram_tensor("xb", [DX, N], f32, kind=kind_big)
    wqt = nc.dram_tensor("wqt", [DX, DQ], f32, kind=kind_big)
    wkt = nc.dram_tensor("wkt", [DX, DKH], f32, kind=kind_big)
    wvt = nc.dram_tensor("wvt", [DX, DKH], f32, kind=kind_big)
    # tiny input consumed into one output element (value 0 at rest): lets a
    # benchmark chain data dependencies between repeated NEFF executions
    seed = nc.dram_tensor("seed", [1, 1], f32, kind="ExternalInput")
    out = nc.dram_tensor("out", [DQ, N], f32, kind=kind_out)
    sink = nc.dram_tensor("sink", [1, 1], f32, kind="ExternalOutput") if bench else None

    xv = xb.ap().rearrange("(dt p) n -> p dt n", p=P)
    wqv = wqt.ap().rearrange("(dt p) q -> p dt q", p=P)
    wkv = wkt.ap().rearrange("(dt p) k -> p dt k", p=P)
    wvv = wvt.ap().rearrange("(dt p) k -> p dt k", p=P)

    with tile.TileContext(nc) as tc:
        with (
            tc.tile_pool(name="dram", bufs=1, space="DRAM") as dram,
            tc.tile_pool(name="ps", bufs=8, space="PSUM") as ps,
        ):
            # DRAM spill tensors
            qth_d = dram.tile([N, DQ], bf16)
            qtl_d = dram.tile([N, DQ], bf16)
            kth_d = dram.tile([N, DKH], bf16)
            ktl_d = dram.tile([N, DKH], bf16)
            vsp_d = dram.tile([DKH, N], f32)

            rep_cm = tc.For_i(0, bench_reps, 1) if bench_reps else None
            if rep_cm is not None:
                rep_cm.__enter__()

            # ---------------- Phase 0 + 1: projections ----------------
            with (
                tc.tile_pool(name="pw", bufs=1) as pw,
                tc.tile_pool(name="pwstage", bufs=2) as pwstage,
                tc.tile_pool(name="px", bufs=2) as px,
                tc.tile_pool(name="pev", bufs=2) as pev,
            ):
                # --- weight prep ---
                if qk_mode == "f32r":
                    wq_h = pw.tile([P, DT, DQ], f32r, tag="wqh")
                    wk_h = pw.tile([P, DT, DKH], f32r, tag="wkh")
                    wq_l = wk_l = None
                else:
                    wq_h = pw.tile([P, DT, DQ], f32r, tag="wqh")
                    wq_l = pw.tile([P, DT, DQ], f32r, tag="wql")
                    wk_h = pw.tile([P, DT, DKH], f32r, tag="wkh")
                    wk_l = pw.tile([P, DT, DKH], f32r, tag="wkl")
                wv_r = pw.tile([P, DT, DKH], f32r, tag="wvr")

                for dt in range(DT):
                    wtmp = pwstage.tile([P, DQ], f32, tag="wtmp")
                    nc.sync.dma_start(wtmp[:], wqv[:, dt])
                    nc.vector.tensor_copy(wq_h[:, dt], wtmp[:])
                    if wq_l is not None:
                        nc.vector.tensor_sub(wq_l[:, dt], wtmp[:], wq_h[:, dt])

                    wtmp2 = pwstage.tile([P, DKH], f32, tag="wtmp2")
                    nc.sync.dma_start(wtmp2[:], wkv[:, dt])
                    nc.vector.tensor_copy(wk_h[:, dt], wtmp2[:])
                    if wk_l is not None:
                        nc.vector.tensor_sub(wk_l[:, dt], wtmp2[:], wk_h[:, dt])

                    wtmp3 = pwstage.tile([P, DKH], f32, tag="wtmp3")
                    nc.sync.dma_start(wtmp3[:], wvv[:, dt])
                    nc.vector.tensor_copy(wv_r[:, dt], wtmp3[:])

                # --- x chunks: project ---
                for c in range(NT512):
                    ncol = bass.ds(c * 512, 512)
                    xc = px.tile([P, DT, 512], f32, tag="xc")
                    nc.sync.dma_start(xc[:], xv[:, :, ncol])
                    xr = px.tile([P, DT, 512], f32r, tag="xr")
                    nc.vector.tensor_copy(xr[:], xc[:])
                    if qk_mode == "bothsplit":
                        xl = px.tile([P, DT, 512], f32r, tag="xl")
                        nc.vector.tensor_sub(xl[:], xc[:], xr[:])

                    # V projection: psum [v-128, n-512]
                    for vt in range(KT):
                        vps = ps.tile([P, 512], f32, tag="ps")
                        vsl = bass.ds(vt * P, P)
                        for dt in range(DT):
                            nc.tensor.matmul(
                                vps[:], wv_r[:, dt, vsl], xr[:, dt],
                                start=(dt == 0), stop=(dt == DT - 1),
                            )
                        vsb = pev.tile([P, 512], f32, tag="vsb")
                        nc.vector.tensor_copy(vsb[:], vps[:])
                        nc.sync.dma_start(vsp_d[vt * P:(vt + 1) * P, ncol], vsb[:])

                    # QT / KT projections per n-subtile
                    for nt in range(4):
                        gnt = c * 4 + nt   # global n-128 tile
                        xsl = bass.ds(nt * P, P)
                        for qc in range(QC):
                            qsl = bass.ds(qc * QCS, QCS)
                            qps = ps.tile([P, QCS], f32, tag="ps")
                            first = True
                            for dt in range(DT):
                                if qk_mode == "f32r":
                                    nc.tensor.matmul(
                                        qps[:], xr[:, dt, xsl], wq_h[:, dt, qsl],
                                        start=first, stop=(dt == DT - 1))
                                    first = False
                                elif qk_mode == "wsplit":
                                    nc.tensor.matmul(
                                        qps[:], xr[:, dt, xsl], wq_h[:, dt, qsl],
                                        start=first, stop=False)
                                    nc.tensor.matmul(
                                        qps[:], xr[:, dt, xsl], wq_l[:, dt, qsl],
                                        start=False, stop=(dt == DT - 1))
                                    first = False
                                else:  # bothsplit
                                    nc.tensor.matmul(
                                        qps[:], xr[:, dt, xsl], wq_h[:, dt, qsl],
                                        start=first, stop=False)
                                    nc.tensor.matmul(
                                        qps[:], xr[:, dt, xsl], wq_l[:, dt, qsl],
                                        start=False, stop=False)
                                    nc.tensor.matmul(
                                        qps[:], xl[:, dt, xsl], wq_h[:, dt, qsl],
                                        start=False, stop=(dt == DT - 1))
                                    first = False
                            qh = pev.tile([P, QCS], bf16, tag="qh")
                            ql = pev.tile([P, QCS], bf16, tag="ql")
                            nc.vector.tensor_copy(qh[:], qps[:])
                            nc.vector.tensor_sub(ql[:], qps[:], qh[:])
                            rows = bass.ds(gnt * P, P)
                            nc.sync.dma_start(qth_d[rows, qsl], qh[:])
                            nc.sync.dma_start(qtl_d[rows, qsl], ql[:])

                        kps = ps.tile([P, DKH], f32, tag="ps")
                        first = True
                        for dt in range(DT):
                            if qk_mode == "f32r":
                                nc.tensor.matmul(
                                    kps[:], xr[:, dt, xsl], wk_h[:, dt],
                                    start=first, stop=(dt == DT - 1))
                                first = False
                            elif qk_mode == "wsplit":
                                nc.tensor.matmul(
                                    kps[:], xr[:, dt, xsl], wk_h[:, dt],
                                    start=first, stop=False)
                                nc.tensor.matmul(
                                    kps[:], xr[:, dt, xsl], wk_l[:, dt],
                                    start=False, stop=(dt == DT - 1))
                                first = False
                            else:
                                nc.tensor.matmul(
                                    kps[:], xr[:, dt, xsl], wk_h[:, dt],
                                    start=first, stop=False)
                                nc.tensor.matmul(
                                    kps[:], xr[:, dt, xsl], wk_l[:, dt],
                                    start=False, stop=False)
                                nc.tensor.matmul(
                                    kps[:], xl[:, dt, xsl], wk_h[:, dt],
                                    start=False, stop=(dt == DT - 1))
                                first = False
                        kh = pev.tile([P, DKH], bf16, tag="kh")
                        kl = pev.tile([P, DKH], bf16, tag="kl")
                        nc.vector.tensor_copy(kh[:], kps[:])
                        nc.vector.tensor_sub(kl[:], kps[:], kh[:])
                        rows = bass.ds(gnt * P, P)
                        nc.sync.dma_start(kth_d[rows, :], kh[:])
                        nc.sync.dma_start(ktl_d[rows, :], kl[:])

            # ---------------- Phase 2: scoresT + softmax ----------------
            pres_cm = tc.tile_pool(name="resident", bufs=1)
            pres = pres_cm.__enter__()
            scores_sb = [pres.tile([P, DQ], f32, tag=f"sc{kt}", name=f"scores{kt}") for kt in range(KT)]
            p_r = [pres.tile([P, DQ], f32r, tag=f"pr{kt}", name=f"p{kt}") for kt in range(KT)]

            with (
                tc.tile_pool(name="pstream", bufs=3) as pstream,
                tc.tile_pool(name="psmx", bufs=2) as psmx,
                tc.tile_pool(name="pstat", bufs=2) as pstat,
            ):
                for qc in range(QC):
                    qsl = bass.ds(qc * QCS, QCS)
                    s_ps = [ps.tile([P, QCS], f32, tag="ps", name=f"sps{qc}_{i}") for i in range(KT)]
                    for nt in range(NT128):
                        rows = bass.ds(nt * P, P)
                        qh_s = pstream.tile([P, QCS], bf16, tag="qh_s")
                        ql_s = pstream.tile([P, QCS], bf16, tag="ql_s")
                        kh_s = pstream.tile([P, DKH], bf16, tag="kh_s")
                        kl_s = pstream.tile([P, DKH], bf16, tag="kl_s")
                        nc.sync.dma_start(qh_s[:], qth_d[rows, qsl])
                        nc.sync.dma_start(ql_s[:], qtl_d[rows, qsl])
                        nc.sync.dma_start(kh_s[:], kth_d[rows, :])
                        nc.sync.dma_start(kl_s[:], ktl_d[rows, :])
                        for kt in range(KT):
                            ksl = bass.ds(kt * P, P)
                            nc.tensor.matmul(
                                s_ps[kt][:], kh_s[:, ksl], qh_s[:],
                                start=(nt == 0), stop=False)
                            nc.tensor.matmul(
                                s_ps[kt][:], kh_s[:, ksl], ql_s[:],
                                start=False, stop=False)
                            nc.tensor.matmul(
                                s_ps[kt][:], kl_s[:, ksl], qh_s[:],
                                start=False, stop=(nt == NT128 - 1))
                    for kt in range(KT):
                        nc.vector.tensor_copy(scores_sb[kt][:, qsl], s_ps[kt][:])

                # softmax over q (free axis) per k row; fold in 1/sqrt(DQ)
                for kt in range(KT):
                    m = pstat.tile([P, 1], f32, tag="m")
                    negm = pstat.tile([P, 1], f32, tag="negm")
                    den = pstat.tile([P, 1], f32, tag="den")
                    rden = pstat.tile([P, 1], f32, tag="rden")
                    nc.vector.reduce_max(m[:], scores_sb[kt][:],
                                         axis=mybir.AxisListType.X)
                    nc.vector.tensor_scalar_mul(negm[:], m[:], -scale)
                    e = psmx.tile([P, DQ], f32, tag="e")
                    nc.scalar.activation(
                        e[:], scores_sb[kt][:],
                        mybir.ActivationFunctionType.Exp,
                        bias=negm[:], scale=scale, accum_out=den[:])
                    nc.vector.reciprocal(rden[:], den[:])
                    nc.vector.tensor_scalar_mul(p_r[kt][:], e[:], rden[:])

            # ---------------- Phase 3: out = pT.T @ V ----------------
            with (
                tc.tile_pool(name="pvin", bufs=KT + 2) as pvin,
                tc.tile_pool(name="pout", bufs=4) as pout,
                tc.tile_pool(name="pseed", bufs=1) as pseed,
            ):
                seed_sb = pseed.tile([1, 1], f32, tag="seed")
                nc.sync.dma_start(seed_sb[:], seed.ap())
                for c in range(NT512):
                    ncol = bass.ds(c * 512, 512)
                    vrr = []
                    for kt in range(KT):
                        vf = pvin.tile([P, 512], f32, tag="vf")
                        nc.sync.dma_start(vf[:], vsp_d[kt * P:(kt + 1) * P, ncol])
                        vr = pvin.tile([P, 512], f32r, tag="vr")
                        nc.vector.tensor_copy(vr[:], vf[:])
                        vrr.append(vr)
                    for qt in range(QT128):
                        ops = ps.tile([P, 512], f32, tag="ps")
                        qsl2 = bass.ds(qt * P, P)
                        for kt in range(KT):
                            nc.tensor.matmul(
                                ops[:], p_r[kt][:, qsl2], vrr[kt][:],
                                start=(kt == 0), stop=(kt == KT - 1))
                        osb = pout.tile([P, 512], f32, tag="osb")
                        nc.vector.tensor_copy(osb[:], ops[:])
                        if c == 0 and qt == 0:
                            nc.vector.tensor_scalar_add(
                                osb[0:1, 0:1], ops[0:1, 0:1], seed_sb[:])
                            if sink is not None:
                                nc.sync.dma_start(sink.ap(), osb[0:1, 0:1])
                        nc.sync.dma_start(out.ap()[qt * P:(qt + 1) * P, ncol], osb[:])
            pres_cm.__exit__(None, None, None)
            if rep_cm is not None:
                rep_cm.__exit__(None, None, None)

    nc.compile()
    return nc


_CACHE = {}


def _get_nc(DX, N, DQ, DKH, qk_mode=QK_PROJ_MODE):
    key = (DX, N, DQ, DKH, qk_mode)
    if key not in _CACHE:
        _CACHE[key] = _build_core_kernel(DX, N, DQ, DKH, qk_mode)
    return _CACHE[key]


def _run(x, Wq, Wk, Wv, **spmd_kwargs):
    """Run the SPMD kernel; returns (out, BassKernelResults)."""
    from concourse.bass_utils import run_bass_kernel_spmd

    B, DX, N = x.shape
    DQ = Wq.shape[0]
    DK = Wk.shape[0]
    assert (B, DX, N, DQ, DK) == (B_FULL, DX_FULL, N_FULL, DQ_FULL, DK_FULL)
    DKH = DK // 2

    nc = _get_nc(DX, N, DQ, DKH)

    WqT = np.ascontiguousarray(Wq.T, dtype=np.float32)
    WkT = np.ascontiguousarray(Wk.T, dtype=np.float32)
    WvT = np.ascontiguousarray(Wv.T, dtype=np.float32)

    in_maps = []
    for c in range(N_CORES):
        b, h = divmod(c, 2)
        hsl = slice(h * DKH, (h + 1) * DKH)
        in_maps.append({
            "xb": np.ascontiguousarray(x[b], dtype=np.float32),
            "wqt": WqT,
            "wkt": np.ascontiguousarray(WkT[:, hsl]),
            "wvt": np.ascontiguousarray(WvT[:, hsl]),
            "seed": np.zeros((1, 1), np.float32),
        })

    res = run_bass_kernel_spmd(nc, in_maps, core_ids=list(range(N_CORES)),
                               **spmd_kwargs)
    out = np.empty((B, DQ, N), np.float32)
    for b in range(B):
        out[b] = res.results[2 * b]["out"] + res.results[2 * b + 1]["out"]
    return out, res


def kernel(x, Wq, Wk, Wv):
    return _run(x, Wq, Wk, Wv)[0]


# revision 17
# speedup vs baseline: 6.2000x; 1.0820x over previous
"""TRN2 Bass kernel for nn_Attention_369367187796.

Reference computation (B=4, DX=1024, N=4096, DQ=DK=DV=1024, fp32):
    Q = Wq @ x[b]; K = Wk @ x[b]; V = Wv @ x[b]          (per batch)
    scores = Q @ K.T   (contract n)
    p = softmax(scores / sqrt(DQ), axis=q)               <- softmax over q!
    out[q,n] = sum_k p[q,k] V[k,n]

Sharding: 8 cores = 4 batches x 2 dk-halves. Each core computes, for its
(batch b, k-half h): the full Q, its half of K and V, scoresT[k_half, q]
(softmax over q is the free axis -> fully local), and the partial
out[q, n] = sum_{k in half} p[k,q] V[k,n]. Host sums the two partials.

Precision strategy (HW-validated, end-to-end rel err ~4e-4 vs fp64):
  - float32r (fp32 rounded to 11 mantissa bits) matmuls run at full PE rate;
    native fp32 runs at 1/4 rate; bf16 alone flips softmax argmaxes (logits
    have std ~740 with top-2 gaps down to ~0.4).
  - Q/K projections: W split into f32r hi+lo (2 passes) - W's positive mean
    makes its rounding error coherent over the d-contraction; x single f32r.
  - scores: Q evicted as f32r hi+lo, K single f32r -> 2-pass split matmul.
  - V projection and p@V: single-pass f32r (errors pass through softmax
    un-amplified).

Layouts (per core):
  QT (n, q) and KT (n, k) are computed transposed so the scores matmul
  contracts n on partitions and softmax lands on the free axis:
    QT[n,q] = sum_d x[d,n] WqT[d,q]   lhsT = x-tile [d,n], rhs = WqT [d,q]
    scoresT[k,q]: lhsT = KT [n,k], rhs = QT [n,q]
    out[q,n]:     lhsT = pT [k,q],  rhs = V  [k,n]
  QT/KT/V spill to DRAM between phases as per-row-tile tensors (fine-grained
  deps let phase 2 start while phase 1 is still projecting later n-tiles).
  The walrus verifier requires f32r matmul operands to be produced by a
  rounding compute op (not DMA), so reloaded spills are re-rounded with a
  cheap f32r->f32r tensor_copy.
"""

import math

import numpy as np

B_FULL, DX_FULL, N_FULL = 4, 1024, 4096
DQ_FULL = DK_FULL = 1024
N_CORES = 8

# precision mode for the Q/K projections: "f32r" (1 pass), "wsplit" (2)
QK_PROJ_MODE = "wsplit"


def _build_core_kernel(DX, N, DQ, DKH, qk_mode=QK_PROJ_MODE, bench=False,
                       bench_reps=0):
    import concourse.bass as bass
    import concourse.mybir as mybir
    import concourse.tile as tile
    from concourse import bacc

    f32 = mybir.dt.float32
    f32r = mybir.dt.float32r

    P = 128
    DT = DX // P            # d-tiles (projection contraction)
    NT512 = N // 512        # n chunks of 512
    NT128 = N // P          # n tiles of 128
    QC = (DQ + 511) // 512  # q chunks of <=512
    QCS = min(DQ, 512)
    KT = DKH // P           # k tiles of 128
    QT128 = DQ // P         # q tiles (out partitions)
    scale = 1.0 / math.sqrt(DQ)

    assert DX % P == 0 and N % 512 == 0 and DQ % P == 0 and DKH % P == 0
    assert DQ % QCS == 0

    nc = bacc.Bacc(None, target_bir_lowering=False, debug=False)

    kind_big = "Internal" if bench else "ExternalInput"
    kind_out = "Internal" if bench else "ExternalOutput"
    xb = nc.dram_tensor("xb", [DX, N], f32, kind=kind_big)
    wqt = nc.dram_tensor("wqt", [DX, DQ], f32, kind=kind_big)
    wkt = nc.dram_tensor("wkt", [DX, DKH], f32, kind=kind_big)
    wvt = nc.dram_tensor("wvt", [DX, DKH], f32, kind=kind_big)
    # tiny input consumed into one output element (value 0 at rest): lets a
    # benchmark chain data dependencies between repeated NEFF executions
    seed = nc.dram_tensor("seed", [1, 1], f32, kind="ExternalInput")
    out = nc.dram_tensor("out", [DQ, N], f32, kind=kind_out)
    sink = (nc.dram_tensor("sink", [1, 1], f32, kind="ExternalOutput")
            if bench else None)

    xv = xb.ap().rearrange("(dt p) n -> p dt n", p=P)
    wqv = wqt.ap().rearrange("(dt p) q -> p dt q", p=P)
    wkv = wkt.ap().rearrange("(dt p) k -> p dt k", p=P)
    wvv = wvt.ap().rearrange("(dt p) k -> p dt k", p=P)

    with tile.TileContext(nc) as tc:
        with (
            tc.tile_pool(name="dram", bufs=1, space="DRAM") as dram,
            tc.tile_pool(name="ps", bufs=8, space="PSUM") as ps,
        ):
            # spills grouped by 4 n-tiles: fine-grained cross-phase deps with
            # batched (1MB-class) reload DMAs. Views: row = t*128 + p.
            NG = NT128 // 4
            qh_d = [dram.tile([4 * P, DQ], f32r, name=f"qh_d{i}").rearrange(
                "(t p) q -> p t q", p=P) for i in range(NG)]
            ql_d = [dram.tile([4 * P, DQ], f32r, name=f"ql_d{i}").rearrange(
                "(t p) q -> p t q", p=P) for i in range(NG)]
            kr_d = [dram.tile([4 * P, DKH], f32r, name=f"kr_d{i}").rearrange(
                "(t p) k -> p t k", p=P) for i in range(NG)]
            v_d = dram.tile([DKH, N], f32r, name="v_d").rearrange(
                "(t p) n -> p t n", p=P)

            rep_cm = tc.For_i(0, bench_reps, 1) if bench_reps else None
            if rep_cm is not None:
                rep_cm.__enter__()

            # ---------------- Phase 0 + 1: projections ----------------
            with (
                tc.tile_pool(name="pw", bufs=1) as pw,
                tc.tile_pool(name="pwstage", bufs=1) as pwstage,
                tc.tile_pool(name="px", bufs=3) as px,
                tc.tile_pool(name="pev", bufs=2) as pev,
            ):
                # --- weight prep: round/split W to f32r in SBUF ---
                if qk_mode == "f32r":
                    wq_h = pw.tile([P, DT, DQ], f32r, tag="wqh")
                    wk_h = pw.tile([P, DT, DKH], f32r, tag="wkh")
                    wq_l = wk_l = None
                else:
                    wq_h = pw.tile([P, DT, DQ], f32r, tag="wqh")
                    wq_l = pw.tile([P, DT, DQ], f32r, tag="wql")
                    wk_h = pw.tile([P, DT, DKH], f32r, tag="wkh")
                    wk_l = pw.tile([P, DT, DKH], f32r, tag="wkl")
                wv_r = pw.tile([P, DT, DKH], f32r, tag="wvr")

                for dt in range(0, DT, 2):
                    d2 = bass.ds(dt, 2)
                    wtmp = pwstage.tile([P, 2, DQ], f32, tag="wtmp")
                    nc.sync.dma_start(wtmp[:], wqv[:, d2])
                    nc.vector.tensor_copy(wq_h[:, d2], wtmp[:])
                    if wq_l is not None:
                        nc.vector.tensor_sub(wq_l[:, d2], wtmp[:], wq_h[:, d2])

                    wtmp2 = pwstage.tile([P, 2, DKH], f32, tag="wtmp2")
                    nc.sync.dma_start(wtmp2[:], wkv[:, d2])
                    nc.vector.tensor_copy(wk_h[:, d2], wtmp2[:])
                    if wk_l is not None:
                        nc.vector.tensor_sub(wk_l[:, d2], wtmp2[:], wk_h[:, d2])

                    wtmp3 = pwstage.tile([P, 2, DKH], f32, tag="wtmp3")
                    nc.sync.dma_start(wtmp3[:], wvv[:, d2])
                    nc.vector.tensor_copy(wv_r[:, d2], wtmp3[:])

                # --- x chunks: project ---
                for c in range(NT512):
                    ncol = bass.ds(c * 512, 512)
                    xc = px.tile([P, DT, 512], f32, tag="x", name=f"xc{c}")
                    nc.sync.dma_start(xc[:], xv[:, :, ncol])
                    xr = px.tile([P, DT, 512], f32r, tag="x", name=f"xr{c}")
                    nc.vector.tensor_copy(xr[:], xc[:])

                    # V projection: psum [v-128, n-512]
                    for vt in range(KT):
                        vps = ps.tile([P, 512], f32, tag="ps", name=f"vps{c}_{vt}")
                        vsl = bass.ds(vt * P, P)
                        for dt in range(DT):
                            nc.tensor.matmul(
                                vps[:], wv_r[:, dt, vsl], xr[:, dt],
                                start=(dt == 0), stop=(dt == DT - 1),
                            )
                        vsb = pev.tile([P, 512], f32r, tag="vsb")
                        nc.vector.tensor_copy(vsb[:], vps[:])
                        nc.scalar.dma_start(v_d[:, vt, ncol], vsb[:])

                    # QT / KT projections per n-subtile
                    for nt in range(4):
                        gnt = c * 4 + nt   # global n-128 tile
                        xsl = bass.ds(nt * P, P)
                        for qc in range(QC):
                            qsl = bass.ds(qc * QCS, QCS)
                            qps = ps.tile([P, QCS], f32, tag="ps",
                                          name=f"qps{gnt}_{qc}")
                            for dt in range(DT):
                                if qk_mode == "f32r":
                                    nc.tensor.matmul(
                                        qps[:], xr[:, dt, xsl], wq_h[:, dt, qsl],
                                        start=(dt == 0), stop=(dt == DT - 1))
                                else:
                                    nc.tensor.matmul(
                                        qps[:], xr[:, dt, xsl], wq_h[:, dt, qsl],
                                        start=(dt == 0), stop=False)
                                    nc.tensor.matmul(
                                        qps[:], xr[:, dt, xsl], wq_l[:, dt, qsl],
                                        start=False, stop=(dt == DT - 1))
                            qh = pev.tile([P, QCS], f32r, tag="qh")
                            ql = pev.tile([P, QCS], f32r, tag="ql")
                            nc.vector.tensor_copy(qh[:], qps[:])
                            nc.vector.tensor_sub(ql[:], qps[:], qh[:])
                            nc.scalar.dma_start(qh_d[gnt // 4][:, gnt % 4, qsl], qh[:])
                            nc.scalar.dma_start(ql_d[gnt // 4][:, gnt % 4, qsl], ql[:])

                        kps = ps.tile([P, DKH], f32, tag="ps", name=f"kps{gnt}")
                        for dt in range(DT):
                            if qk_mode == "f32r":
                                nc.tensor.matmul(
                                    kps[:], xr[:, dt, xsl], wk_h[:, dt],
                                    start=(dt == 0), stop=(dt == DT - 1))
                            else:
                                nc.tensor.matmul(
                                    kps[:], xr[:, dt, xsl], wk_h[:, dt],
                                    start=(dt == 0), stop=False)
                                nc.tensor.matmul(
                                    kps[:], xr[:, dt, xsl], wk_l[:, dt],
                                    start=False, stop=(dt == DT - 1))
                        kr = pev.tile([P, DKH], f32r, tag="kr")
                        nc.vector.tensor_copy(kr[:], kps[:])
                        nc.scalar.dma_start(kr_d[gnt // 4][:, gnt % 4], kr[:])

            # ---------------- Phase 2: scoresT + softmax ----------------
            pres_cm = tc.tile_pool(name="resident", bufs=1)
            pres = pres_cm.__enter__()
            scores_sb = [pres.tile([P, DQ], f32, tag=f"sc{kt}", name=f"scores{kt}")
                         for kt in range(KT)]
            p_r = [pres.tile([P, DQ], f32r, tag=f"pr{kt}", name=f"p{kt}")
                   for kt in range(KT)]

            with (
                tc.tile_pool(name="pstream", bufs=3) as pstream,
                tc.tile_pool(name="psmx", bufs=2) as psmx,
                tc.tile_pool(name="pstat", bufs=2) as pstat,
            ):
                for qc in range(QC):
                    qsl = bass.ds(qc * QCS, QCS)
                    s_ps = [ps.tile([P, QCS], f32, tag="ps", name=f"sps{qc}_{i}")
                            for i in range(KT)]
                    for g in range(NG):
                        qh_s = pstream.tile([P, 4, QCS], f32r, tag="qh_s")
                        ql_s = pstream.tile([P, 4, QCS], f32r, tag="ql_s")
                        kr_s = pstream.tile([P, 4, DKH], f32r, tag="kr_s")
                        nc.sync.dma_start(qh_s[:], qh_d[g][:, :, qsl])
                        nc.sync.dma_start(ql_s[:], ql_d[g][:, :, qsl])
                        nc.sync.dma_start(kr_s[:], kr_d[g][:])
                        # in-place re-round after DMA (verifier: f32r matmul
                        # operands need a rounding compute producer)
                        nc.vector.tensor_copy(qh_s[:], qh_s[:])
                        nc.vector.tensor_copy(ql_s[:], ql_s[:])
                        nc.vector.tensor_copy(kr_s[:], kr_s[:])
                        for t in range(4):
                            nt = g * 4 + t
                            for kt in range(KT):
                                ksl = bass.ds(kt * P, P)
                                nc.tensor.matmul(
                                    s_ps[kt][:], kr_s[:, t, ksl], qh_s[:, t],
                                    start=(nt == 0), stop=False)
                                nc.tensor.matmul(
                                    s_ps[kt][:], kr_s[:, t, ksl], ql_s[:, t],
                                    start=False, stop=(nt == NT128 - 1))
                    for kt in range(KT):
                        nc.vector.tensor_copy(scores_sb[kt][:, qsl], s_ps[kt][:])

                # softmax over q (free axis) per k row; fold in 1/sqrt(DQ)
                for kt in range(KT):
                    m = pstat.tile([P, 1], f32, tag="m")
                    negm = pstat.tile([P, 1], f32, tag="negm")
                    den = pstat.tile([P, 1], f32, tag="den")
                    rden = pstat.tile([P, 1], f32, tag="rden")
                    nc.vector.reduce_max(m[:], scores_sb[kt][:],
                                         axis=mybir.AxisListType.X)
                    nc.vector.tensor_scalar_mul(negm[:], m[:], -scale)
                    e = psmx.tile([P, DQ], f32, tag="e")
                    nc.scalar.activation(
                        e[:], scores_sb[kt][:],
                        mybir.ActivationFunctionType.Exp,
                        bias=negm[:], scale=scale, accum_out=den[:])
                    nc.vector.reciprocal(rden[:], den[:])
                    nc.vector.tensor_scalar_mul(p_r[kt][:], e[:], rden[:])

            # ---------------- Phase 3: out = pT.T @ V ----------------
            with (
                tc.tile_pool(name="pvin", bufs=2) as pvin,
                tc.tile_pool(name="pout", bufs=4) as pout,
                tc.tile_pool(name="pseed", bufs=1) as pseed,
            ):
                seed_sb = pseed.tile([1, 1], f32, tag="seed")
                nc.sync.dma_start(seed_sb[:], seed.ap())
                outv = out.ap().rearrange("(qt p) n -> p qt n", p=P)
                for c in range(NT512):
                    ncol = bass.ds(c * 512, 512)
                    vf = pvin.tile([P, KT, 512], f32r, tag="vf")
                    nc.sync.dma_start(vf[:], v_d[:, :, ncol])
                    nc.vector.tensor_copy(vf[:], vf[:])
                    for qg in range(QT128 // 4):
                        osb = pout.tile([P, 4, 512], f32, tag="osb")
                        for qi in range(4):
                            qt = qg * 4 + qi
                            ops = ps.tile([P, 512], f32, tag="ps",
                                          name=f"ops{c}_{qt}")
                            qsl2 = bass.ds(qt * P, P)
                            for kt in range(KT):
                                nc.tensor.matmul(
                                    ops[:], p_r[kt][:, qsl2], vf[:, kt],
                                    start=(kt == 0), stop=(kt == KT - 1))
                            nc.vector.tensor_copy(osb[:, qi], ops[:])
                            if c == 0 and qt == 0:
                                nc.vector.tensor_scalar_add(
                                    osb[0:1, 0, 0:1], ops[0:1, 0:1], seed_sb[:])
                                if sink is not None:
                                    nc.sync.dma_start(sink.ap(), osb[0:1, 0, 0:1])
                        nc.gpsimd.dma_start(
                            outv[:, qg * 4:(qg + 1) * 4, ncol], osb[:])
            pres_cm.__exit__(None, None, None)
            if rep_cm is not None:
                rep_cm.__exit__(None, None, None)

    nc.compile()
    return nc


_CACHE = {}


def _get_nc(DX, N, DQ, DKH, qk_mode=QK_PROJ_MODE):
    key = (DX, N, DQ, DKH, qk_mode)
    if key not in _CACHE:
        _CACHE[key] = _build_core_kernel(DX, N, DQ, DKH, qk_mode)
    return _CACHE[key]


def _run(x, Wq, Wk, Wv, **spmd_kwargs):
    """Run the SPMD kernel; returns (out, BassKernelResults)."""
    from concourse.bass_utils import run_bass_kernel_spmd

    B, DX, N = x.shape
    DQ = Wq.shape[0]
    DK = Wk.shape[0]
    assert (B, DX, N, DQ, DK) == (B_FULL, DX_FULL, N_FULL, DQ_FULL, DK_FULL)
    DKH = DK // 2

    nc = _get_nc(DX, N, DQ, DKH)

    WqT = np.ascontiguousarray(Wq.T, dtype=np.float32)
    WkT = np.ascontiguousarray(Wk.T, dtype=np.float32)
    WvT = np.ascontiguousarray(Wv.T, dtype=np.float32)

    in_maps = []
    for c in range(N_CORES):
        b, h = divmod(c, 2)
        hsl = slice(h * DKH, (h + 1) * DKH)
        in_maps.append({
            "xb": np.ascontiguousarray(x[b], dtype=np.float32),
            "wqt": WqT,
            "wkt": np.ascontiguousarray(WkT[:, hsl]),
            "wvt": np.ascontiguousarray(WvT[:, hsl]),
            "seed": np.zeros((1, 1), np.float32),
        })

    res = run_bass_kernel_spmd(nc, in_maps, core_ids=list(range(N_CORES)),
                               **spmd_kwargs)
    out = np.empty((B, DQ, N), np.float32)
    for b in range(B):
        out[b] = res.results[2 * b]["out"] + res.results[2 * b + 1]["out"]
    return out, res


def kernel(x, Wq, Wk, Wv):
    return _run(x, Wq, Wk, Wv)[0]


# revision 18
# speedup vs baseline: 7.7537x; 1.2506x over previous
"""TRN2 Bass kernel for nn_Attention_369367187796.

Reference computation (B=4, DX=1024, N=4096, DQ=DK=DV=1024, fp32):
    Q = Wq @ x[b]; K = Wk @ x[b]; V = Wv @ x[b]          (per batch)
    scores = Q @ K.T   (contract n)
    p = softmax(scores / sqrt(DQ), axis=q)               <- softmax over q!
    out[q,n] = sum_k p[q,k] V[k,n]

Sharding: 8 cores = 4 batches x 2 dk-halves. Each core computes, for its
(batch b, k-half h): the full Q, its half of K and V, scoresT[k_half, q]
(softmax over q is the free axis -> fully local), and the partial
out[q, n] = sum_{k in half} p[k,q] V[k,n]. Host sums the two partials.

Precision strategy (HW-validated, end-to-end rel err ~4e-4 vs fp64):
  - float32r (fp32 rounded to 11 mantissa bits) matmuls run at full PE rate;
    native fp32 runs at 1/4 rate; bf16 alone flips softmax argmaxes (logits
    have std ~740 with top-2 gaps down to ~0.4).
  - Q/K projections: W split into f32r hi+lo (2 passes) - W's positive mean
    makes its rounding error coherent over the d-contraction; x single f32r.
  - scores: Q evicted as f32r hi+lo, K single f32r -> 2-pass split matmul.
  - V projection and p@V: single-pass f32r (errors pass through softmax
    un-amplified).

Layouts (per core):
  QT (n, q) and KT (n, k) are computed transposed so the scores matmul
  contracts n on partitions and softmax lands on the free axis:
    QT[n,q] = sum_d x[d,n] WqT[d,q]   lhsT = x-tile [d,n], rhs = WqT [d,q]
    scoresT[k,q]: lhsT = KT [n,k], rhs = QT [n,q]
    out[q,n]:     lhsT = pT [k,q],  rhs = V  [k,n]
  QT/KT/V spill to DRAM between phases as per-row-tile tensors (fine-grained
  deps let phase 2 start while phase 1 is still projecting later n-tiles).
  The walrus verifier requires f32r matmul operands to be produced by a
  rounding compute op (not DMA), so reloaded spills are re-rounded with a
  cheap f32r->f32r tensor_copy.
"""

import math

import numpy as np

B_FULL, DX_FULL, N_FULL = 4, 1024, 4096
DQ_FULL = DK_FULL = 1024
N_CORES = 8

# precision mode for the Q/K projections: "f32r" (1 pass), "wsplit" (2)
QK_PROJ_MODE = "wsplit"


def _build_core_kernel(DX, N, DQ, DKH, qk_mode=QK_PROJ_MODE, bench=False,
                       bench_reps=0):
    import concourse.bass as bass
    import concourse.mybir as mybir
    import concourse.tile as tile
    from concourse import bacc

    f32 = mybir.dt.float32
    f32r = mybir.dt.float32r

    P = 128
    DT = DX // P            # d-tiles (projection contraction)
    NT512 = N // 512        # n chunks of 512
    NT128 = N // P          # n tiles of 128
    QC = (DQ + 511) // 512  # q chunks of <=512
    QCS = min(DQ, 512)
    KT = DKH // P           # k tiles of 128
    QT128 = DQ // P         # q tiles (out partitions)
    scale = 1.0 / math.sqrt(DQ)

    assert DX % P == 0 and N % 512 == 0 and DQ % P == 0 and DKH % P == 0
    assert DQ % QCS == 0

    nc = bacc.Bacc(None, target_bir_lowering=False, debug=False)

    kind_big = "Internal" if bench else "ExternalInput"
    kind_out = "Internal" if bench else "ExternalOutput"
    xb = nc.dram_tensor("xb", [DX, N], f32, kind=kind_big)
    wqt = nc.dram_tensor("wqt", [DX, DQ], f32, kind=kind_big)
    wkt = nc.dram_tensor("wkt", [DX, DKH], f32, kind=kind_big)
    wvt = nc.dram_tensor("wvt", [DX, DKH], f32, kind=kind_big)
    # tiny input consumed into one output element (value 0 at rest): lets a
    # benchmark chain data dependencies between repeated NEFF executions
    seed = nc.dram_tensor("seed", [1, 1], f32, kind="ExternalInput")
    out = nc.dram_tensor("out", [DQ, N], f32, kind=kind_out)
    sink = (nc.dram_tensor("sink", [1, 1], f32, kind="ExternalOutput")
            if bench else None)

    xv = xb.ap().rearrange("(dt p) n -> p dt n", p=P)
    wqv = wqt.ap().rearrange("(dt p) q -> p dt q", p=P)
    wkv = wkt.ap().rearrange("(dt p) k -> p dt k", p=P)
    wvv = wvt.ap().rearrange("(dt p) k -> p dt k", p=P)

    with tile.TileContext(nc) as tc:
        with (
            tc.tile_pool(name="dram", bufs=1, space="DRAM") as dram,
            tc.tile_pool(name="ps", bufs=8, space="PSUM") as ps,
        ):
            # spills grouped by 4 n-tiles: fine-grained cross-phase deps with
            # batched (1MB-class) reload DMAs. Views: row = t*128 + p.
            NG = NT128 // 4
            qh_d = [dram.tile([4 * P, DQ], f32r, name=f"qh_d{i}").rearrange(
                "(t p) q -> p t q", p=P) for i in range(NG)]
            ql_d = [dram.tile([4 * P, DQ], f32r, name=f"ql_d{i}").rearrange(
                "(t p) q -> p t q", p=P) for i in range(NG)]
            kr_d = [dram.tile([4 * P, DKH], f32r, name=f"kr_d{i}").rearrange(
                "(t p) k -> p t k", p=P) for i in range(NG)]
            v_d = dram.tile([DKH, N], f32r, name="v_d").rearrange(
                "(t p) n -> p t n", p=P)

            rep_cm = tc.For_i(0, bench_reps, 1) if bench_reps else None
            if rep_cm is not None:
                rep_cm.__enter__()

            # ---------------- Phase 0 + 1: projections ----------------
            with (
                tc.tile_pool(name="pw", bufs=1) as pw,
                tc.tile_pool(name="pwstage", bufs=1) as pwstage,
                tc.tile_pool(name="px", bufs=3) as px,
                tc.tile_pool(name="pev", bufs=2) as pev,
            ):
                # --- weight prep: round/split W to f32r in SBUF ---
                if qk_mode == "f32r":
                    wq_h = pw.tile([P, DT, DQ], f32r, tag="wqh")
                    wk_h = pw.tile([P, DT, DKH], f32r, tag="wkh")
                    wq_l = wk_l = None
                else:
                    wq_h = pw.tile([P, DT, DQ], f32r, tag="wqh")
                    wq_l = pw.tile([P, DT, DQ], f32r, tag="wql")
                    # K projection runs single-pass f32r: K-rounding error is
                    # largely common-mode within each softmax row (err 6.4e-4
                    # vs 4.5e-4 end-to-end, CPU-verified on the real inputs)
                    wk_h = pw.tile([P, DT, DKH], f32r, tag="wkh")
                    wk_l = None
                wv_r = pw.tile([P, DT, DKH], f32r, tag="wvr")

                pre_xc = px.tile([P, DT, 512], f32, tag="x", name="xc0")
                nc.sync.dma_start(pre_xc[:], xv[:, :, bass.ds(0, 512)])

                for dt in range(0, DT, 2):
                    d2 = bass.ds(dt, 2)
                    wtmp = pwstage.tile([P, 2, DQ], f32, tag="wtmp")
                    nc.sync.dma_start(wtmp[:], wqv[:, d2])
                    nc.vector.tensor_copy(wq_h[:, d2], wtmp[:])
                    if wq_l is not None:
                        nc.vector.tensor_sub(wq_l[:, d2], wtmp[:], wq_h[:, d2])

                    wtmp2 = pwstage.tile([P, 2, DKH], f32, tag="wtmp2")
                    nc.sync.dma_start(wtmp2[:], wkv[:, d2])
                    nc.vector.tensor_copy(wk_h[:, d2], wtmp2[:])
                    if wk_l is not None:
                        nc.vector.tensor_sub(wk_l[:, d2], wtmp2[:], wk_h[:, d2])

                    wtmp3 = pwstage.tile([P, 2, DKH], f32, tag="wtmp3")
                    nc.sync.dma_start(wtmp3[:], wvv[:, d2])
                    nc.vector.tensor_copy(wv_r[:, d2], wtmp3[:])

                # --- x chunks: project ---
                for c in range(NT512):
                    ncol = bass.ds(c * 512, 512)
                    if c == 0:
                        xc = pre_xc
                    else:
                        xc = px.tile([P, DT, 512], f32, tag="x", name=f"xc{c}")
                        nc.sync.dma_start(xc[:], xv[:, :, ncol])
                    xr = px.tile([P, DT, 512], f32r, tag="x", name=f"xr{c}")
                    nc.vector.tensor_copy(xr[:], xc[:])

                    # V projection: psum [v-128, n-512]
                    for vt in range(KT):
                        vps = ps.tile([P, 512], f32, tag="ps", name=f"vps{c}_{vt}")
                        vsl = bass.ds(vt * P, P)
                        for dt in range(DT):
                            nc.tensor.matmul(
                                vps[:], wv_r[:, dt, vsl], xr[:, dt],
                                start=(dt == 0), stop=(dt == DT - 1),
                            )
                        vsb = pev.tile([P, 512], f32r, tag="vsb")
                        nc.vector.tensor_copy(vsb[:], vps[:])
                        nc.scalar.dma_start(v_d[:, vt, ncol], vsb[:])

                    # QT / KT projections per n-subtile
                    for nt in range(4):
                        gnt = c * 4 + nt   # global n-128 tile
                        xsl = bass.ds(nt * P, P)
                        for qc in range(QC):
                            qsl = bass.ds(qc * QCS, QCS)
                            qps = ps.tile([P, QCS], f32, tag="ps",
                                          name=f"qps{gnt}_{qc}")
                            for dt in range(DT):
                                if qk_mode == "f32r":
                                    nc.tensor.matmul(
                                        qps[:], xr[:, dt, xsl], wq_h[:, dt, qsl],
                                        start=(dt == 0), stop=(dt == DT - 1))
                                else:
                                    nc.tensor.matmul(
                                        qps[:], xr[:, dt, xsl], wq_h[:, dt, qsl],
                                        start=(dt == 0), stop=False)
                                    nc.tensor.matmul(
                                        qps[:], xr[:, dt, xsl], wq_l[:, dt, qsl],
                                        start=False, stop=(dt == DT - 1))
                            qh = pev.tile([P, QCS], f32r, tag="qh")
                            ql = pev.tile([P, QCS], f32r, tag="ql")
                            nc.vector.tensor_copy(qh[:], qps[:])
                            nc.vector.tensor_sub(ql[:], qps[:], qh[:])
                            nc.scalar.dma_start(qh_d[gnt // 4][:, gnt % 4, qsl], qh[:])
                            nc.scalar.dma_start(ql_d[gnt // 4][:, gnt % 4, qsl], ql[:])

                        kps = ps.tile([P, DKH], f32, tag="ps", name=f"kps{gnt}")
                        for dt in range(DT):
                            nc.tensor.matmul(
                                kps[:], xr[:, dt, xsl], wk_h[:, dt],
                                start=(dt == 0), stop=(dt == DT - 1))
                        kr = pev.tile([P, DKH], f32r, tag="kr")
                        nc.vector.tensor_copy(kr[:], kps[:])
                        nc.scalar.dma_start(kr_d[gnt // 4][:, gnt % 4], kr[:])

            # ---------------- Phase 2: scoresT + softmax ----------------
            pres_cm = tc.tile_pool(name="resident", bufs=1)
            pres = pres_cm.__enter__()
            scores_sb = [pres.tile([P, DQ], f32, tag=f"sc{kt}", name=f"scores{kt}")
                         for kt in range(KT)]
            p_r = [pres.tile([P, DQ], f32r, tag=f"pr{kt}", name=f"p{kt}")
                   for kt in range(KT)]

            with (
                tc.tile_pool(name="pstream", bufs=3) as pstream,
                tc.tile_pool(name="psmx", bufs=2) as psmx,
                tc.tile_pool(name="pstat", bufs=2) as pstat,
            ):
                for qc in range(QC):
                    qsl = bass.ds(qc * QCS, QCS)
                    s_ps = [ps.tile([P, QCS], f32, tag="ps", name=f"sps{qc}_{i}")
                            for i in range(KT)]
                    for g in range(NG):
                        qh_s = pstream.tile([P, 4, QCS], f32r, tag="qh_s")
                        ql_s = pstream.tile([P, 4, QCS], f32r, tag="ql_s")
                        kr_s = pstream.tile([P, 4, DKH], f32r, tag="kr_s")
                        nc.sync.dma_start(qh_s[:], qh_d[g][:, :, qsl])
                        nc.sync.dma_start(ql_s[:], ql_d[g][:, :, qsl])
                        nc.sync.dma_start(kr_s[:], kr_d[g][:])
                        # in-place re-round after DMA (verifier: f32r matmul
                        # operands need a rounding compute producer)
                        nc.vector.tensor_copy(qh_s[:], qh_s[:])
                        nc.vector.tensor_copy(ql_s[:], ql_s[:])
                        nc.vector.tensor_copy(kr_s[:], kr_s[:])
                        for t in range(4):
                            nt = g * 4 + t
                            for kt in range(KT):
                                ksl = bass.ds(kt * P, P)
                                nc.tensor.matmul(
                                    s_ps[kt][:], kr_s[:, t, ksl], qh_s[:, t],
                                    start=(nt == 0), stop=False)
                                nc.tensor.matmul(
                                    s_ps[kt][:], kr_s[:, t, ksl], ql_s[:, t],
                                    start=False, stop=(nt == NT128 - 1))
                    for kt in range(KT):
                        nc.vector.tensor_copy(scores_sb[kt][:, qsl], s_ps[kt][:])

                # softmax over q (free axis) per k row; fold in 1/sqrt(DQ)
                for kt in range(KT):
                    m = pstat.tile([P, 1], f32, tag="m")
                    negm = pstat.tile([P, 1], f32, tag="negm")
                    den = pstat.tile([P, 1], f32, tag="den")
                    rden = pstat.tile([P, 1], f32, tag="rden")
                    nc.vector.reduce_max(m[:], scores_sb[kt][:],
                                         axis=mybir.AxisListType.X)
                    nc.vector.tensor_scalar_mul(negm[:], m[:], -scale)
                    e = psmx.tile([P, DQ], f32, tag="e")
                    nc.scalar.activation(
                        e[:], scores_sb[kt][:],
                        mybir.ActivationFunctionType.Exp,
                        bias=negm[:], scale=scale, accum_out=den[:])
                    nc.vector.reciprocal(rden[:], den[:])
                    nc.vector.tensor_scalar_mul(p_r[kt][:], e[:], rden[:])

            # ---------------- Phase 3: out = pT.T @ V ----------------
            with (
                tc.tile_pool(name="pvin", bufs=2) as pvin,
                tc.tile_pool(name="pout", bufs=4) as pout,
                tc.tile_pool(name="pseed", bufs=1) as pseed,
            ):
                seed_sb = pseed.tile([1, 1], f32, tag="seed")
                nc.sync.dma_start(seed_sb[:], seed.ap())
                outv = out.ap().rearrange("(qt p) n -> p qt n", p=P)
                for c in range(NT512):
                    ncol = bass.ds(c * 512, 512)
                    vf = pvin.tile([P, KT, 512], f32r, tag="vf")
                    nc.sync.dma_start(vf[:], v_d[:, :, ncol])
                    nc.vector.tensor_copy(vf[:], vf[:])
                    for qg in range(QT128 // 4):
                        osb = pout.tile([P, 4, 512], f32, tag="osb")
                        for qi in range(4):
                            qt = qg * 4 + qi
                            ops = ps.tile([P, 512], f32, tag="ps",
                                          name=f"ops{c}_{qt}")
                            qsl2 = bass.ds(qt * P, P)
                            for kt in range(KT):
                                nc.tensor.matmul(
                                    ops[:], p_r[kt][:, qsl2], vf[:, kt],
                                    start=(kt == 0), stop=(kt == KT - 1))
                            nc.vector.tensor_copy(osb[:, qi], ops[:])
                            if c == 0 and qt == 0:
                                nc.vector.tensor_scalar_add(
                                    osb[0:1, 0, 0:1], ops[0:1, 0:1], seed_sb[:])
                                if sink is not None:
                                    nc.sync.dma_start(sink.ap(), osb[0:1, 0, 0:1])
                        nc.gpsimd.dma_start(
                            outv[:, qg * 4:(qg + 1) * 4, ncol], osb[:])
            pres_cm.__exit__(None, None, None)
            if rep_cm is not None:
                rep_cm.__exit__(None, None, None)

    nc.compile()
    return nc


_CACHE = {}


def _get_nc(DX, N, DQ, DKH, qk_mode=QK_PROJ_MODE):
    key = (DX, N, DQ, DKH, qk_mode)
    if key not in _CACHE:
        _CACHE[key] = _build_core_kernel(DX, N, DQ, DKH, qk_mode)
    return _CACHE[key]


def _run(x, Wq, Wk, Wv, **spmd_kwargs):
    """Run the SPMD kernel; returns (out, BassKernelResults)."""
    from concourse.bass_utils import run_bass_kernel_spmd

    B, DX, N = x.shape
    DQ = Wq.shape[0]
    DK = Wk.shape[0]
    assert (B, DX, N, DQ, DK) == (B_FULL, DX_FULL, N_FULL, DQ_FULL, DK_FULL)
    DKH = DK // 2

    nc = _get_nc(DX, N, DQ, DKH)

    WqT = np.ascontiguousarray(Wq.T, dtype=np.float32)
    WkT = np.ascontiguousarray(Wk.T, dtype=np.float32)
    WvT = np.ascontiguousarray(Wv.T, dtype=np.float32)

    in_maps = []
    for c in range(N_CORES):
        b, h = divmod(c, 2)
        hsl = slice(h * DKH, (h + 1) * DKH)
        in_maps.append({
            "xb": np.ascontiguousarray(x[b], dtype=np.float32),
            "wqt": WqT,
            "wkt": np.ascontiguousarray(WkT[:, hsl]),
            "wvt": np.ascontiguousarray(WvT[:, hsl]),
            "seed": np.zeros((1, 1), np.float32),
        })

    res = run_bass_kernel_spmd(nc, in_maps, core_ids=list(range(N_CORES)),
                               **spmd_kwargs)
    out = np.empty((B, DQ, N), np.float32)
    for b in range(B):
        out[b] = res.results[2 * b]["out"] + res.results[2 * b + 1]["out"]
    return out, res


def kernel(x, Wq, Wk, Wv):
    return _run(x, Wq, Wk, Wv)[0]


# revision 19
# speedup vs baseline: 8.7711x; 1.1312x over previous
"""TRN2 Bass kernel for nn_Attention_369367187796.

Reference computation (B=4, DX=1024, N=4096, DQ=DK=DV=1024, fp32):
    Q = Wq @ x[b]; K = Wk @ x[b]; V = Wv @ x[b]          (per batch)
    scores = Q @ K.T   (contract n)
    p = softmax(scores / sqrt(DQ), axis=q)               <- softmax over q!
    out[q,n] = sum_k p[q,k] V[k,n]

Sharding: 8 cores = 4 batches x 2 dk-halves. Each core computes, for its
(batch b, k-half h): the full Q, its half of K and V, scoresT[k_half, q]
(softmax over q is the free axis -> fully local), and the partial
out[q, n] = sum_{k in half} p[k,q] V[k,n]. Host sums the two partials.

Precision strategy (HW-validated, end-to-end rel err ~4e-4 vs fp64):
  - float32r (fp32 rounded to 11 mantissa bits) matmuls run at full PE rate;
    native fp32 runs at 1/4 rate; bf16 alone flips softmax argmaxes (logits
    have std ~740 with top-2 gaps down to ~0.4).
  - Q/K projections: W split into f32r hi+lo (2 passes) - W's positive mean
    makes its rounding error coherent over the d-contraction; x single f32r.
  - scores: Q evicted as f32r hi+lo, K single f32r -> 2-pass split matmul.
  - V projection and p@V: single-pass f32r (errors pass through softmax
    un-amplified).

Layouts (per core):
  QT (n, q) and KT (n, k) are computed transposed so the scores matmul
  contracts n on partitions and softmax lands on the free axis:
    QT[n,q] = sum_d x[d,n] WqT[d,q]   lhsT = x-tile [d,n], rhs = WqT [d,q]
    scoresT[k,q]: lhsT = KT [n,k], rhs = QT [n,q]
    out[q,n]:     lhsT = pT [k,q],  rhs = V  [k,n]
  QT/KT/V spill to DRAM between phases as per-row-tile tensors (fine-grained
  deps let phase 2 start while phase 1 is still projecting later n-tiles).
  The walrus verifier requires f32r matmul operands to be produced by a
  rounding compute op (not DMA), so reloaded spills are re-rounded with a
  cheap f32r->f32r tensor_copy.
"""

import math

import numpy as np

B_FULL, DX_FULL, N_FULL = 4, 1024, 4096
DQ_FULL = DK_FULL = 1024
N_CORES = 8

# precision mode for the Q/K projections: "f32r" (1 pass), "wsplit" (2)
QK_PROJ_MODE = "wsplit"


def _build_core_kernel(DX, N, DQ, DKH, qk_mode=QK_PROJ_MODE, bench=False,
                       bench_reps=0):
    import concourse.bass as bass
    import concourse.mybir as mybir
    import concourse.tile as tile
    from concourse import bacc

    f32 = mybir.dt.float32
    f32r = mybir.dt.float32r

    P = 128
    DT = DX // P            # d-tiles (projection contraction)
    NT512 = N // 512        # n chunks of 512
    NT128 = N // P          # n tiles of 128
    QC = (DQ + 511) // 512  # q chunks of <=512
    QCS = min(DQ, 512)
    KT = DKH // P           # k tiles of 128
    QT128 = DQ // P         # q tiles (out partitions)
    scale = 1.0 / math.sqrt(DQ)

    assert DX % P == 0 and N % 512 == 0 and DQ % P == 0 and DKH % P == 0
    assert DQ % QCS == 0

    nc = bacc.Bacc(None, target_bir_lowering=False, debug=False)

    kind_big = "Internal" if bench else "ExternalInput"
    kind_out = "Internal" if bench else "ExternalOutput"
    xb = nc.dram_tensor("xb", [DX, N], f32, kind=kind_big)
    wqt = nc.dram_tensor("wqt", [DX, DQ], f32, kind=kind_big)
    wkt = nc.dram_tensor("wkt", [DX, DKH], f32, kind=kind_big)
    wvt = nc.dram_tensor("wvt", [DX, DKH], f32, kind=kind_big)
    # tiny input consumed into one output element (value 0 at rest): lets a
    # benchmark chain data dependencies between repeated NEFF executions
    seed = nc.dram_tensor("seed", [1, 1], f32, kind="ExternalInput")
    out = nc.dram_tensor("out", [DQ, N], f32, kind=kind_out)
    sink = (nc.dram_tensor("sink", [1, 1], f32, kind="ExternalOutput")
            if bench else None)

    xv = xb.ap().rearrange("(dt p) n -> p dt n", p=P)
    wqv = wqt.ap().rearrange("(dt p) q -> p dt q", p=P)
    wkv = wkt.ap().rearrange("(dt p) k -> p dt k", p=P)
    wvv = wvt.ap().rearrange("(dt p) k -> p dt k", p=P)

    with tile.TileContext(nc) as tc:
        with (
            tc.tile_pool(name="dram", bufs=1, space="DRAM") as dram,
            tc.tile_pool(name="ps", bufs=8, space="PSUM") as ps,
        ):
            # spills grouped by 4 n-tiles: fine-grained cross-phase deps with
            # batched (1MB-class) reload DMAs. Views: row = t*128 + p.
            NG = NT128 // 4
            qh_d = [dram.tile([4 * P, DQ], f32r, name=f"qh_d{i}").rearrange(
                "(t p) q -> p t q", p=P) for i in range(NG)]
            ql_d = [dram.tile([4 * P, DQ], f32r, name=f"ql_d{i}").rearrange(
                "(t p) q -> p t q", p=P) for i in range(NG)]
            kr_d = [dram.tile([4 * P, DKH], f32r, name=f"kr_d{i}").rearrange(
                "(t p) k -> p t k", p=P) for i in range(NG)]
            v_d = dram.tile([DKH, N], f32r, name="v_d").rearrange(
                "(t p) n -> p t n", p=P)

            rep_cm = tc.For_i(0, bench_reps, 1) if bench_reps else None
            if rep_cm is not None:
                rep_cm.__enter__()

            # ---------------- Phase 0 + 1: projections ----------------
            with (
                tc.tile_pool(name="pw", bufs=1) as pw,
                tc.tile_pool(name="pwstage", bufs=1) as pwstage,
                tc.tile_pool(name="px", bufs=3) as px,
                tc.tile_pool(name="pev", bufs=2) as pev,
            ):
                # --- weight prep: round W to f32r in SBUF ---
                # The host passes Wq/Wk MINUS 0.5 (zero-mean entries): their
                # f32r rounding error is then incoherent over the
                # d-contraction, so Q and K projections run single-pass f32r.
                # The exact mean term 0.5*colsum(x)[n] is restored with one
                # K=1 matmul per projection psum; its residual rounding error
                # is constant across q and cancels exactly in the softmax.
                wq_h = pw.tile([P, DT, DQ], f32r, tag="wqh")
                wk_h = pw.tile([P, DT, DKH], f32r, tag="wkh")
                wq_l = wk_l = None
                wv_r = pw.tile([P, DT, DKH], f32r, tag="wvr")

                ones_c = pw.tile([P, 1], f32r, tag="ones")
                half_row = pw.tile([1, 512], f32r, tag="half")
                cstage = pwstage.tile([P, 512], f32, tag="cstage")
                nc.gpsimd.memset(cstage[:], 1.0)
                nc.vector.tensor_copy(ones_c[:], cstage[:, 0:1])
                nc.gpsimd.memset(cstage[:, 0:512], 0.5)
                nc.vector.tensor_copy(half_row[:], cstage[0:1, 0:512])

                pre_xc = px.tile([P, DT, 512], f32, tag="x", name="xc0")
                nc.sync.dma_start(pre_xc[:], xv[:, :, bass.ds(0, 512)])

                for dt in range(0, DT, 2):
                    d2 = bass.ds(dt, 2)
                    wtmp = pwstage.tile([P, 2, DQ], f32, tag="wtmp")
                    nc.sync.dma_start(wtmp[:], wqv[:, d2])
                    nc.vector.tensor_copy(wq_h[:, d2], wtmp[:])

                    wtmp2 = pwstage.tile([P, 2, DKH], f32, tag="wtmp2")
                    nc.sync.dma_start(wtmp2[:], wkv[:, d2])
                    nc.vector.tensor_copy(wk_h[:, d2], wtmp2[:])

                    wtmp3 = pwstage.tile([P, 2, DKH], f32, tag="wtmp3")
                    nc.sync.dma_start(wtmp3[:], wvv[:, d2])
                    nc.vector.tensor_copy(wv_r[:, d2], wtmp3[:])

                # --- x chunks: project ---
                for c in range(NT512):
                    ncol = bass.ds(c * 512, 512)
                    if c == 0:
                        xc = pre_xc
                    else:
                        xc = px.tile([P, DT, 512], f32, tag="x", name=f"xc{c}")
                        nc.sync.dma_start(xc[:], xv[:, :, ncol])
                    xr = px.tile([P, DT, 512], f32r, tag="x", name=f"xr{c}")
                    nc.vector.tensor_copy(xr[:], xc[:])

                    # s[n] = colsum_d x (from xr): ones-matmul, [1, 512]
                    sps = ps.tile([P, 512], f32, tag="ps", name=f"sps_c{c}")
                    for dt in range(DT):
                        nc.tensor.matmul(sps[0:1, :], ones_c[:], xr[:, dt],
                                         start=(dt == 0), stop=(dt == DT - 1))
                    s_sb = pev.tile([1, 512], f32r, tag="s_sb")
                    nc.vector.tensor_copy(s_sb[:], sps[0:1, :])

                    # V projection: psum [v-128, n-512]
                    for vt in range(KT):
                        vps = ps.tile([P, 512], f32, tag="ps", name=f"vps{c}_{vt}")
                        vsl = bass.ds(vt * P, P)
                        for dt in range(DT):
                            nc.tensor.matmul(
                                vps[:], wv_r[:, dt, vsl], xr[:, dt],
                                start=(dt == 0), stop=(dt == DT - 1),
                            )
                        vsb = pev.tile([P, 512], f32r, tag="vsb")
                        nc.vector.tensor_copy(vsb[:], vps[:])
                        nc.scalar.dma_start(v_d[:, vt, ncol], vsb[:])

                    # QT / KT projections per n-subtile
                    for nt in range(4):
                        gnt = c * 4 + nt   # global n-128 tile
                        xsl = bass.ds(nt * P, P)
                        for qc in range(QC):
                            qsl = bass.ds(qc * QCS, QCS)
                            qps = ps.tile([P, QCS], f32, tag="ps",
                                          name=f"qps{gnt}_{qc}")
                            for dt in range(DT):
                                nc.tensor.matmul(
                                    qps[:], xr[:, dt, xsl], wq_h[:, dt, qsl],
                                    start=(dt == 0), stop=False)
                            nc.tensor.matmul(
                                qps[:], s_sb[0:1, xsl], half_row[:, :QCS],
                                start=False, stop=True)
                            qh = pev.tile([P, QCS], f32r, tag="qh")
                            ql = pev.tile([P, QCS], f32r, tag="ql")
                            nc.vector.tensor_copy(qh[:], qps[:])
                            nc.vector.tensor_sub(ql[:], qps[:], qh[:])
                            nc.scalar.dma_start(qh_d[gnt // 4][:, gnt % 4, qsl], qh[:])
                            nc.scalar.dma_start(ql_d[gnt // 4][:, gnt % 4, qsl], ql[:])

                        kps = ps.tile([P, DKH], f32, tag="ps", name=f"kps{gnt}")
                        for dt in range(DT):
                            nc.tensor.matmul(
                                kps[:], xr[:, dt, xsl], wk_h[:, dt],
                                start=(dt == 0), stop=False)
                        nc.tensor.matmul(
                            kps[:], s_sb[0:1, xsl], half_row[:, :DKH],
                            start=False, stop=True)
                        kr = pev.tile([P, DKH], f32r, tag="kr")
                        nc.vector.tensor_copy(kr[:], kps[:])
                        nc.scalar.dma_start(kr_d[gnt // 4][:, gnt % 4], kr[:])

            # ---------------- Phase 2: scoresT + softmax ----------------
            pres_cm = tc.tile_pool(name="resident", bufs=1)
            pres = pres_cm.__enter__()
            scores_sb = [pres.tile([P, DQ], f32, tag=f"sc{kt}", name=f"scores{kt}")
                         for kt in range(KT)]
            p_r = [pres.tile([P, DQ], f32r, tag=f"pr{kt}", name=f"p{kt}")
                   for kt in range(KT)]

            with (
                tc.tile_pool(name="pstream", bufs=3) as pstream,
                tc.tile_pool(name="psmx", bufs=2) as psmx,
                tc.tile_pool(name="pstat", bufs=2) as pstat,
            ):
                for qc in range(QC):
                    qsl = bass.ds(qc * QCS, QCS)
                    s_ps = [ps.tile([P, QCS], f32, tag="ps", name=f"sps{qc}_{i}")
                            for i in range(KT)]
                    for g in range(NG):
                        qh_s = pstream.tile([P, 4, QCS], f32r, tag="qh_s")
                        ql_s = pstream.tile([P, 4, QCS], f32r, tag="ql_s")
                        kr_s = pstream.tile([P, 4, DKH], f32r, tag="kr_s")
                        nc.sync.dma_start(qh_s[:], qh_d[g][:, :, qsl])
                        nc.sync.dma_start(ql_s[:], ql_d[g][:, :, qsl])
                        nc.sync.dma_start(kr_s[:], kr_d[g][:])
                        # in-place re-round after DMA (verifier: f32r matmul
                        # operands need a rounding compute producer)
                        nc.vector.tensor_copy(qh_s[:], qh_s[:])
                        nc.vector.tensor_copy(ql_s[:], ql_s[:])
                        nc.vector.tensor_copy(kr_s[:], kr_s[:])
                        for t in range(4):
                            nt = g * 4 + t
                            for kt in range(KT):
                                ksl = bass.ds(kt * P, P)
                                nc.tensor.matmul(
                                    s_ps[kt][:], kr_s[:, t, ksl], qh_s[:, t],
                                    start=(nt == 0), stop=False)
                                nc.tensor.matmul(
                                    s_ps[kt][:], kr_s[:, t, ksl], ql_s[:, t],
                                    start=False, stop=(nt == NT128 - 1))
                    for kt in range(KT):
                        nc.vector.tensor_copy(scores_sb[kt][:, qsl], s_ps[kt][:])

                # softmax over q (free axis) per k row; fold in 1/sqrt(DQ)
                for kt in range(KT):
                    m = pstat.tile([P, 1], f32, tag="m")
                    negm = pstat.tile([P, 1], f32, tag="negm")
                    den = pstat.tile([P, 1], f32, tag="den")
                    rden = pstat.tile([P, 1], f32, tag="rden")
                    nc.vector.reduce_max(m[:], scores_sb[kt][:],
                                         axis=mybir.AxisListType.X)
                    nc.vector.tensor_scalar_mul(negm[:], m[:], -scale)
                    e = psmx.tile([P, DQ], f32, tag="e")
                    nc.scalar.activation(
                        e[:], scores_sb[kt][:],
                        mybir.ActivationFunctionType.Exp,
                        bias=negm[:], scale=scale, accum_out=den[:])
                    nc.vector.reciprocal(rden[:], den[:])
                    nc.vector.tensor_scalar_mul(p_r[kt][:], e[:], rden[:])

            # ---------------- Phase 3: out = pT.T @ V ----------------
            with (
                tc.tile_pool(name="pvin", bufs=2) as pvin,
                tc.tile_pool(name="pout", bufs=4) as pout,
                tc.tile_pool(name="pseed", bufs=1) as pseed,
            ):
                seed_sb = pseed.tile([1, 1], f32, tag="seed")
                nc.sync.dma_start(seed_sb[:], seed.ap())
                outv = out.ap().rearrange("(qt p) n -> p qt n", p=P)
                for c in range(NT512):
                    ncol = bass.ds(c * 512, 512)
                    vf = pvin.tile([P, KT, 512], f32r, tag="vf")
                    nc.sync.dma_start(vf[:], v_d[:, :, ncol])
                    nc.vector.tensor_copy(vf[:], vf[:])
                    for qg in range(QT128 // 4):
                        osb = pout.tile([P, 4, 512], f32, tag="osb")
                        for qi in range(4):
                            qt = qg * 4 + qi
                            ops = ps.tile([P, 512], f32, tag="ps",
                                          name=f"ops{c}_{qt}")
                            qsl2 = bass.ds(qt * P, P)
                            for kt in range(KT):
                                nc.tensor.matmul(
                                    ops[:], p_r[kt][:, qsl2], vf[:, kt],
                                    start=(kt == 0), stop=(kt == KT - 1))
                            nc.vector.tensor_copy(osb[:, qi], ops[:])
                            if c == 0 and qt == 0:
                                nc.vector.tensor_scalar_add(
                                    osb[0:1, 0, 0:1], ops[0:1, 0:1], seed_sb[:])
                                if sink is not None:
                                    nc.sync.dma_start(sink.ap(), osb[0:1, 0, 0:1])
                        nc.gpsimd.dma_start(
                            outv[:, qg * 4:(qg + 1) * 4, ncol], osb[:])
            pres_cm.__exit__(None, None, None)
            if rep_cm is not None:
                rep_cm.__exit__(None, None, None)

    nc.compile()
    return nc


_CACHE = {}


def _get_nc(DX, N, DQ, DKH, qk_mode=QK_PROJ_MODE):
    key = (DX, N, DQ, DKH, qk_mode)
    if key not in _CACHE:
        _CACHE[key] = _build_core_kernel(DX, N, DQ, DKH, qk_mode)
    return _CACHE[key]


def _run(x, Wq, Wk, Wv, **spmd_kwargs):
    """Run the SPMD kernel; returns (out, BassKernelResults)."""
    from concourse.bass_utils import run_bass_kernel_spmd

    B, DX, N = x.shape
    DQ = Wq.shape[0]
    DK = Wk.shape[0]
    assert (B, DX, N, DQ, DK) == (B_FULL, DX_FULL, N_FULL, DQ_FULL, DK_FULL)
    DKH = DK // 2

    nc = _get_nc(DX, N, DQ, DKH)

    # Wq/Wk are shipped mean-removed (entries - 0.5); the kernel restores
    # the exact 0.5*colsum(x) term on-chip (see builder comment)
    WqT = np.ascontiguousarray(Wq.T, dtype=np.float32) - np.float32(0.5)
    WkT = np.ascontiguousarray(Wk.T, dtype=np.float32) - np.float32(0.5)
    WvT = np.ascontiguousarray(Wv.T, dtype=np.float32)

    in_maps = []
    for c in range(N_CORES):
        b, h = divmod(c, 2)
        hsl = slice(h * DKH, (h + 1) * DKH)
        in_maps.append({
            "xb": np.ascontiguousarray(x[b], dtype=np.float32),
            "wqt": WqT,
            "wkt": np.ascontiguousarray(WkT[:, hsl]),
            "wvt": np.ascontiguousarray(WvT[:, hsl]),
            "seed": np.zeros((1, 1), np.float32),
        })

    res = run_bass_kernel_spmd(nc, in_maps, core_ids=list(range(N_CORES)),
                               **spmd_kwargs)
    out = np.empty((B, DQ, N), np.float32)
    for b in range(B):
        out[b] = res.results[2 * b]["out"] + res.results[2 * b + 1]["out"]
    return out, res


def kernel(x, Wq, Wk, Wv):
    return _run(x, Wq, Wk, Wv)[0]
